# revision 1
# baseline (speedup 1.0000x reference)
"""Bass/Trainium2 kernel for per-chunk fake-quant + linear.

reference semantics (per chunk c):
    q  = clip(round(x/s_c), -128, 127) * s_c
    out[c] = q @ w[c].T          # [B,S,O]

Strategy (per-core HW time ~182us vs ~165us HBM roofline at the measured
~420 GB/s per-NeuronCore rate; 68MB of traffic/core):
  - Data-parallel over tokens: each of 8 cores gets T = B*S/8 = 8192 tokens
    (all 4 chunks), weights replicated.
  - Host pre-transposes each x shard to [C, D, T] so the contraction dim d
    sits on SBUF partitions (contiguous 4KB-run DMA loads), and un-permutes
    the output ([C, 128, T/128, O] on device -> 8KB-contiguous store runs).
  - Integer-domain matmul: qi = clip(rne(x*1/s), -128, 127) is an integer in
    [-128,127], exactly representable in f16 -> full-rate f16 matmuls.
    The scale s is folded into the weights host-side (ws = s*w) and
    pre-scaled by 2^10 to keep all f16 weights normal; the 2^-10 dequant is
    folded into the PSUM->SBUF copy scale. Weight f16 rounding = ~1e-4 rel.
  - Rounding uses the HW f32->int32 convert, verified on-device to be
    round-to-nearest-even, matching jnp.round (CoreSim truncates instead -
    hardware is truth).
  - Engine balance: convert pass split ACT/DVE, int clip on DVE, PSUM
    copies ~4.5:3.5 ACT:DVE, in-DMAs on the sync HWDGE ring, out-DMAs split
    scalar HWDGE + gpsimd SWDGE, weights on SWDGE. All engines stay under
    the DMA pole; HBM is saturated wall-to-wall at ~420 GB/s.
"""

import numpy as np
import ml_dtypes

import concourse.bass as bass
import concourse.tile as tile
import concourse.mybir as mybir
from concourse.bass_utils import run_bass_kernel_spmd
def _split_sync_waits(nc):
    """Hoist excess per-instruction sem waits onto preceding same-engine NOPs.

    This walrus build rejects instructions carrying >2 sync waits ("Too many
    sync wait commands", CoreV2/V3GenImpl setupSyncWait). A NOP on the same
    engine immediately before the instruction blocks the queue identically,
    so semantics are preserved.
    """
    count = 0
    for fn in nc.m.functions:
        for bb in fn.blocks:
            out = []
            for ins in bb.instructions:
                si = ins.sync_info
                waits = list(si.on_wait) if (si and si.on_wait) else []
                maxw = 1
                if len(waits) > maxw:
                    extra, keep = waits[:-maxw], waits[-maxw:]
                    ins.sync_info = mybir.SyncInfo(
                        on_wait=keep, on_update=list(si.on_update or [])
                    )
                    for j in range(0, len(extra), maxw):
                        count += 1
                        nop = mybir.InstNoOp(
                            name=f"ant-waitsplit-{count}", ins=[], outs=[]
                        )
                        nop.engine = ins.engine
                        nop.sync_info = mybir.SyncInfo(
                            on_wait=extra[j : j + maxw], on_update=[]
                        )
                        out.append(nop)
                out.append(ins)
            bb.instructions = out
    return count

C, B, S, D, O = 4, 8, 8192, 256, 256
NCORES = 8
N = B * S            # tokens per chunk (65536)
T = N // NCORES      # tokens per chunk per core (8192)

WS_SHIFT = 10           # weights pre-scaled by 2^10 to stay f16-normal
DEQUANT = float(2.0 ** -WS_SHIFT)


def _build_program(scales, t_kern=T, tt=1024):
    """Build the SPMD Bass program (same program on all cores).

    Inputs (per core): xt [C, D, t_kern] f32, wh/wl [C, D, O] bf16.
    Output: out [C, t_kern, O] f32.
    """
    f32 = mybir.dt.float32
    f16 = mybir.dt.float16
    i32 = mybir.dt.int32
    alu = mybir.AluOpType

    assert t_kern % tt == 0 and tt % 128 == 0
    n_tt = t_kern // tt
    n_s4 = tt // 128

    nc = bass.Bass()
    xt = nc.declare_dram_parameter("xt", [C, D, t_kern], f32, isOutput=False)
    # ws16 = (s*w).T * 2^WS_SHIFT as f16 (integers*f16 weights at full PE
    # rate; 2^WS_SHIFT keeps all weights in f16 normal range)
    ws16 = nc.declare_dram_parameter("ws16", [C, D, O], f16, isOutput=False)
    # Permuted output layout: out_dev[c, p, j, o] = out[c, j*128 + p, o].
    # Partition p's DMA runs are then (n_s4*O*4)=8KB contiguous instead of 1KB.
    out = nc.declare_dram_parameter(
        "out", [C, 128, t_kern // 128, O], f32, isOutput=True
    )

    with tile.TileContext(nc) as tc:
        with (
            tc.tile_pool(name="wpool", bufs=1) as wpool,
            tc.tile_pool(name="xpool", bufs=8) as xpool,
            tc.tile_pool(name="t1pool", bufs=3) as t1pool,
            tc.tile_pool(name="qpool", bufs=6) as qpool,
            tc.tile_pool(name="opool", bufs=6) as opool,
            tc.tile_pool(name="ppool", bufs=8, space=bass.MemorySpace.PSUM) as ppool,
        ):
            # Resident weights: wsT[c][dk], each [128, O] f16. One DMA, on
            # the SWDGE ring so the HWDGE rings start streaming x at once.
            wt = {}
            w_tile = wpool.tile([128, 2 * C * O], f16, tag="w")
            nc.gpsimd.dma_start(
                out=w_tile[:].rearrange("p (g o) -> p g o", o=O),
                in_=ws16[:].rearrange("c (dk p) o -> p (c dk) o", p=128),
            )
            for c in range(C):
                for dk in range(2):
                    g = c * 2 + dk
                    wt[c, dk] = w_tile[:, g * O : (g + 1) * O]

            for c in range(C):
                inv_s = float(np.float32(1.0) / np.float32(scales[c]))
                for it in range(n_tt):
                    in_eng = nc.sync
                    # Load x tile: [p=128 (d%128), (dk, t)] from xt[c]
                    x_tile = xpool.tile([128, 2 * tt], f32, tag="x")
                    src = xt[c].rearrange("(dk p) t -> p dk t", dk=2)[
                        :, :, it * tt : (it + 1) * tt
                    ]
                    dst = x_tile[:].rearrange("p (dk t) -> p dk t", dk=2)
                    in_eng.dma_start(out=dst, in_=src)

                    # k = rne(x * inv_s) via the HW f32->int32 convert (RNE,
                    # verified on-device to match jnp.round half-to-even).
                    # Split across ACT and DVE - the convert runs 1x on both.
                    t1 = t1pool.tile([128, 2 * tt], i32, tag="t1")
                    nc.scalar.mul(t1[:, : tt], x_tile[:, : tt], inv_s)
                    nc.vector.tensor_scalar(
                        t1[:, tt :], x_tile[:, tt :], inv_s, None, alu.mult
                    )
                    # qi = clip(k, -128, 127) as f16 (int32->f16 exact here)
                    qi = qpool.tile([128, 2 * tt], f16, tag="qi")
                    nc.vector.tensor_scalar(
                        qi[:], t1[:], -128, 127, alu.max, alu.min
                    )

                    # Matmuls: out[t0:t0+128, :] = qi_tile.T @ wsT
                    stage = opool.tile([128, n_s4 * O], f32, tag="stage")
                    for s4 in range(n_s4):
                        ps = ppool.tile([128, O], f32, tag="ps")
                        for dk in range(2):
                            lhsT = qi[:, dk * tt + s4 * 128 : dk * tt + s4 * 128 + 128]
                            nc.tensor.matmul(
                                ps[:], lhsT, wt[c, dk],
                                start=(dk == 0), stop=(dk == 1),
                            )
                        # PSUM -> SBUF staging with the 2^-WS_SHIFT dequant
                        # folded in (ACT ~4.5 of 8, DVE rest)
                        dst = stage[:, s4 * O : (s4 + 1) * O]
                        act_copy = (s4 % 2 == 0) or (s4 == 1 and it % 2 == 0)
                        if act_copy:
                            nc.scalar.mul(dst, ps[:], DEQUANT)
                        else:
                            nc.vector.tensor_scalar(
                                dst, ps[:], DEQUANT, None, alu.mult
                            )

                    # Store tt tokens: stage [p, (s4, o)] -> out[c, it*tt + s4*128 + p, o]
                    # Split the store across both out rings (scalar HWDGE +
                    # gpsimd SWDGE) so each drains half every tile.
                    half = n_s4 // 2
                    stv = stage[:].rearrange("p (s4 o) -> p s4 o", o=O)
                    nc.scalar.dma_start(
                        out=out[c][:, it * n_s4 : it * n_s4 + half, :],
                        in_=stv[:, :half, :],
                    )
                    nc.gpsimd.dma_start(
                        out=out[c][:, it * n_s4 + half : (it + 1) * n_s4, :],
                        in_=stv[:, half:, :],
                    )
    return nc


def _prep_inputs(x, w, scales, t_kern=T, ncores=NCORES):
    x = np.ascontiguousarray(np.asarray(x, dtype=np.float32)).reshape(C, N, D)
    w = np.asarray(w, dtype=np.float32)
    s = np.asarray(scales, dtype=np.float32).reshape(C, 1, 1)

    ws = s * w                                            # [C, O, D] f32
    wsT = np.ascontiguousarray(ws.transpose(0, 2, 1))     # [C, D, O]
    ws16 = (wsT * np.float32(2.0**WS_SHIFT)).astype(np.float16)

    in_maps = []
    for i in range(ncores):
        xs = x[:, i * t_kern : (i + 1) * t_kern, :]       # [C, T, D] view
        xtp = np.ascontiguousarray(xs.transpose(0, 2, 1))  # [C, D, T]
        in_maps.append({"xt": xtp, "ws16": ws16})
    return in_maps


def run(x, w, scales, trace=False, **spmd_kwargs):
    """Compile + run on 8 cores. Returns (out, BassKernelResults)."""
    scales = np.asarray(scales, dtype=np.float32)
    nc = _build_program(scales)
    _split_sync_waits(nc)  # HW-only fixup (CoreSim chokes on raw-BIR NoOps)
    in_maps = _prep_inputs(x, w, scales)
    res = run_bass_kernel_spmd(
        nc, in_maps, core_ids=list(range(NCORES)), trace=trace, **spmd_kwargs
    )
    # Un-permute each shard: [C, 128, T/128, O] -> [C, T, O]
    shards = [
        r["out"].transpose(0, 2, 1, 3).reshape(C, T, O) for r in res.results
    ]
    out = np.concatenate(shards, axis=1)                  # [C, N, O]
    return np.ascontiguousarray(out).reshape(C, B, S, O), res


def kernel(x, w, scales):
    out, _ = run(x, w, scales, trace=False)
    return out

